# revision 1
# baseline (speedup 1.0000x reference)
"""Contrastive (MixAware) loss kernel for Trainium2, 8 NeuronCores.

Strategy (replicated keys, zero collectives):
  - x = representations [2B, D] with B=4096, D=256. Rows split into
    queries q = x[:B] and positives p = x[B:].
  - The loss needs the full q-vs-q cosine-similarity matrix. Instead of
    all-gathering normalized embeddings (collective latency dominates at
    this size), every core receives the FULL q block, rotated so its own
    512-row shard sits first: q_core_c = roll(q, -512c). Each core
    redundantly normalizes + transposes all 4096 queries (cheap element
    work) and computes the 512x4096 row-block of exp(sim/T_NEG) for its
    own rows, with fused row-sum accumulation on the Scalar engine.
  - Per-row outputs (partial row sums per column quarter, the diagonal
    correction and the positive-pair cosine sim) are tiny tensors; the
    host does the final log/sum reduction like the row-parallel baseline.
  - Engine split honors the real Pool ISA (no TensorScalar, no PSUM
    access): ssq + PSUM copies on DVE, normalize split DVE/Pool (Pool via
    stride-0-broadcast TensorTensor), transposes on PE via identity into
    PSUM, matmuls on PE, exp + fused row-sum on ACT. The main loop is
    interleaved per column-quarter so DMA, DVE, Pool, PE and ACT all
    overlap, and each quarter's inv batch is emitted before the previous
    quarter's exps so the ACT stream never stalls on prep work.
"""

import numpy as np

import concourse.bass as bass
import concourse.mybir as mybir
import concourse.tile as tile
from concourse import bacc
from concourse.bass_utils import run_bass_kernel_spmd
from concourse.masks import make_identity

B = 4096
D = 256
NCORES = 8
RPC = B // NCORES        # 512 query rows per core
NT = B // 128            # 32 row tiles of the full q block
MT = RPC // 128          # 4 own row tiles
DC = D // 128            # 2 contraction chunks of 128
NQ = 4                   # column quarters of 1024 keys
T_POS = 0.05
T_NEG = 0.1

F32 = mybir.dt.float32
BF16 = mybir.dt.bfloat16
ALU = mybir.AluOpType
ACTF = mybir.ActivationFunctionType


class _Bacc(bacc.Bacc):
    """Bacc that restricts Ln/Exp to the combined natural_log_exp table set so
    interleaved Ln/Exp emit a single ACT table load instead of thrashing."""

    def insert_act_table_loads(self):
        import bass_rust as _bass_rust
        from concourse.hw_specs import get_activation_tables

        has_activation = any(
            isinstance(i, mybir.InstActivation)
            for b in self.main_func.blocks
            for i in b.instructions
        )
        if not has_activation:
            return
        items = list(get_activation_tables(self.m.arch).items())
        lnexp = {ACTF.Ln, ACTF.Exp}
        tables = [
            (k, v if k == "natural_log_exp_and_others" else (v - lnexp))
            for k, v in items
        ]
        _bass_rust.insert_act_table_loads(self, tables)


def _emit_body(nc, tc, pools, rep, q_d, p_d, qsum_d, diag_d, pos_d, ident):
    sb, workv, small = pools

    q_sb = sb.tile([128, NT, D], F32, tag="q_sb")
    p_sb = sb.tile([128, MT, D], F32, tag="p_sb")
    qn_bf = sb.tile([128, NT, D], BF16, tag="qn_bf")
    qt = sb.tile([128, DC, B], BF16, tag="qt")

    ssq = small.tile([128, NT], F32, tag="ssq")
    ln_buf = small.tile([128, NT], F32, tag="ln_buf")
    inv = small.tile([128, NT], F32, tag="inv")
    ssq_p = small.tile([128, MT], F32, tag="ssq_p")
    ln_p = small.tile([128, MT], F32, tag="ln_p")
    inv_p = small.tile([128, MT], F32, tag="inv_p")
    diag_raw = small.tile([128, MT], F32, tag="diag_raw")
    diag_exp = small.tile([128, MT], F32, tag="diag_exp")
    pos_raw = small.tile([128, MT], F32, tag="pos_raw")
    pos_sb = small.tile([128, MT], F32, tag="pos_sb")
    qsum = small.tile([128, MT, NQ], F32, tag="qsum")

    # q arrives rotated so every core's own rows are tiles 0..MT-1.
    # Layout: tile j = 4b+t holds row (512b + 4p + t) on partition p; each
    # partition line is a 4KB-contiguous DRAM read (good descriptors).
    for b in range(NCORES):
        chunk = q_d.ap()[b * RPC : (b + 1) * RPC, :]
        ch_re = chunk.rearrange("(p t) d -> p t d", p=128)
        if b < 2:
            # half-chunk DMAs on the ramp-in path: first data lands ~1us
            # earlier and the first normalize waits on 2 ssq ops, not 4
            nc.sync.dma_start(out=q_sb[:, 4 * b : 4 * b + 2, :],
                              in_=ch_re[:, 0:2, :])
            nc.sync.dma_start(out=q_sb[:, 4 * b + 2 : 4 * b + 4, :],
                              in_=ch_re[:, 2:4, :])
        else:
            nc.sync.dma_start(out=q_sb[:, 4 * b : 4 * b + 4, :], in_=ch_re)
        if b == 3:
            # p is small and only needed by the late extras; slot it after the
            # first quarters' q chunks so it never stalls the prep pipeline
            nc.sync.dma_start(
                out=p_sb[:], in_=p_d.ap().rearrange("(p t) d -> p t d", p=128)
            )

    def prep_ssq_tiles(j0, n):
        # sum of squares per row tile (DVE: Pool has no TensorScalar ISA)
        for j in range(j0, j0 + n):
            scratch = workv.tile([128, D], F32, tag="scr")
            nc.vector.scalar_tensor_tensor(
                out=scratch[:], in0=q_sb[:, j, :], scalar=1.0,
                in1=q_sb[:, j, :], op0=ALU.mult, op1=ALU.mult,
                accum_out=ssq[:, j : j + 1],
            )


    def prep_inv(j0, n, pin=False):
        # inv_norm = exp(-0.5 * ln(ssq))  (Rsqrt activation is disallowed).
        # Ramp-in batches (pin=True) share one ln scratch slice: the WAR
        # hazard stops the scheduler hoisting later ln ops ahead of earlier
        # exps, which stalls the first normalize chain. Mid-stream batches
        # use distinct slices so they schedule as early as data allows.
        s0 = 0 if pin else j0
        nc.scalar.activation(out=ln_buf[:, s0 : s0 + n],
                             in_=ssq[:, j0 : j0 + n], func=ACTF.Ln)
        nc.scalar.activation(out=inv[:, j0 : j0 + n],
                             in_=ln_buf[:, s0 : s0 + n], func=ACTF.Exp,
                             scale=-0.5)

    def prep_mul_pair(ja):
        # normalize to bf16 (DVE/Pool split: DVE owns ssq+copies and is the
        # busiest engine; Pool takes half the muls as TensorTensor with a
        # stride-0 broadcast of inv — TensorScalar is not in the Pool ISA)
        nc.vector.tensor_scalar_mul(
            out=qn_bf[:, ja, :], in0=q_sb[:, ja, :],
            scalar1=inv[:, ja : ja + 1],
        )
        nc.gpsimd.tensor_tensor(
            out=qn_bf[:, ja + 1, :], in0=q_sb[:, ja + 1, :],
            in1=inv[:, ja + 1 : ja + 2].broadcast_to([128, D]),
            op=ALU.mult,
        )

    def prep_transpose_chunk(b, tpsum):
        # transpose a whole chunk via PE into ONE full-bank staging tile,
        # then a single PSUM->SBUF copy on DVE (GPSIMD has no PSUM access).
        # One buffer per chunk means the next chunk's transposes never wait
        # on this chunk's copy (the 2-buffer ring covers two chunks).
        j0 = 4 * b
        pt = tpsum.tile([128, 1024], BF16, tag="pt")
        for dc in range(DC):
            for dj in range(4):
                nc.tensor.transpose(
                    pt[:, (4 * dc + dj) * 128 : (4 * dc + dj + 1) * 128],
                    qn_bf[:, j0 + dj, dc * 128 : (dc + 1) * 128],
                    ident[:],
                )
        nc.vector.tensor_copy(
            out=qt[:, :, j0 * 128 : j0 * 128 + 512],
            in_=pt[:].rearrange("p (dc x) -> p dc x", dc=2),
        )

    def own_extras():
        # diagonal term exp(sim_ii/T_NEG) from the same bf16 values the
        # matmul sees, positive-pair dot products in f32
        for m in range(MT):
            scrb = workv.tile([128, D], BF16, tag="scrb")
            nc.vector.scalar_tensor_tensor(
                out=scrb[:], in0=qn_bf[:, m, :], scalar=1.0,
                in1=qn_bf[:, m, :], op0=ALU.mult, op1=ALU.mult,
                accum_out=diag_raw[:, m : m + 1],
            )
        nc.scalar.activation(out=diag_exp[:], in_=diag_raw[:],
                             func=ACTF.Exp, scale=1.0 / T_NEG)
        for m in range(MT):
            scr = workv.tile([128, D], F32, tag="scr")
            nc.vector.scalar_tensor_tensor(
                out=scr[:], in0=p_sb[:, m, :], scalar=1.0,
                in1=p_sb[:, m, :], op0=ALU.mult, op1=ALU.mult,
                accum_out=ssq_p[:, m : m + 1],
            )
            scr2 = workv.tile([128, D], F32, tag="scr")
            nc.vector.scalar_tensor_tensor(
                out=scr2[:], in0=q_sb[:, m, :], scalar=1.0,
                in1=p_sb[:, m, :], op0=ALU.mult, op1=ALU.mult,
                accum_out=pos_raw[:, m : m + 1],
            )
        nc.scalar.activation(out=ln_p[:], in_=ssq_p[:], func=ACTF.Ln)
        nc.scalar.activation(out=inv_p[:], in_=ln_p[:], func=ACTF.Exp,
                             scale=-0.5)
        nc.vector.tensor_mul(out=pos_sb[:], in0=pos_raw[:], in1=inv[:, 0:MT])
        nc.vector.tensor_mul(out=pos_sb[:], in0=pos_sb[:], in1=inv_p[:])
        nc.sync.dma_start(out=pos_d.ap(), in_=pos_sb[:])
        nc.sync.dma_start(out=diag_d.ap(), in_=diag_exp[:])

    def main_quarter(q, mpsum):
        for m in range(MT):
            ps = mpsum.tile([128, 1024], F32, tag="mm")
            for dc in range(DC):
                for rb in range(2):
                    c0 = q * 1024 + rb * 512
                    nc.tensor.matmul(
                        ps[:, rb * 512 : (rb + 1) * 512],
                        lhsT=qt[:, dc, m * 128 : (m + 1) * 128],
                        rhs=qt[:, dc, c0 : c0 + 512],
                        start=(dc == 0),
                        stop=(dc == DC - 1),
                    )
            nc.scalar.activation(
                out=ps[:], in_=ps[:], func=ACTF.Exp, scale=1.0 / T_NEG,
                accum_out=qsum[:, m, q : q + 1],
            )

    with (
        tc.tile_pool(name=f"tpsum{rep}", bufs=2, space="PSUM") as tpsum,
        tc.tile_pool(name=f"mpsum{rep}", bufs=3, space="PSUM") as mpsum,
    ):
        # Emission order = per-engine program order. Keep the ACT stream lean:
        # each quarter's inv batch is emitted before the PREVIOUS quarter's
        # main exps, so inv never queues behind 4.7us of exp work.
        for q in range(NQ):
            if q == 0:
                # finest-grained ramp-in: per tile-pair ssq -> inv -> muls so
                # the first matmul group's inputs finish as early as DVE's
                # packed schedule allows
                for b in (0, 1):
                    prep_ssq_tiles(4 * b, 2)
                    prep_inv(4 * b, 2, pin=True)
                    prep_mul_pair(4 * b)
                    prep_ssq_tiles(4 * b + 2, 2)
                    prep_inv(4 * b + 2, 2, pin=True)
                    prep_mul_pair(4 * b + 2)
                    prep_transpose_chunk(b, tpsum)
            else:
                prep_ssq_tiles(8 * q, 4)
                prep_ssq_tiles(8 * q + 4, 4)
                prep_inv(8 * q, 8)
                if q == NQ - 1:
                    # extras' ACT ops run here, between inv and the exps of
                    # quarter NQ-2, instead of delaying the final quarter's
                    # exps (and the output tail) by ~0.6us
                    own_extras()
                main_quarter(q - 1, mpsum)
                for b in (2 * q, 2 * q + 1):
                    prep_mul_pair(4 * b)
                    prep_mul_pair(4 * b + 2)
                    prep_transpose_chunk(b, tpsum)
        main_quarter(NQ - 1, mpsum)

    # host does the final denom = sum(qsum) - diag reduction
    nc.sync.dma_start(out=qsum_d.ap(), in_=qsum[:])


def _build(reps=1):
    nc = _Bacc(
        "TRN2", target_bir_lowering=False, debug=False, num_devices=NCORES
    )
    q_d = nc.dram_tensor("q", [B, D], F32, kind="ExternalInput")
    p_d = nc.dram_tensor("p", [RPC, D], F32, kind="ExternalInput")
    qsum_d = nc.dram_tensor("qsum", [128, MT, NQ], F32, kind="ExternalOutput")
    diag_d = nc.dram_tensor("diag", [128, MT], F32, kind="ExternalOutput")
    pos_d = nc.dram_tensor("pos", [128, MT], F32, kind="ExternalOutput")

    with tile.TileContext(nc) as tc:
        with (
            tc.tile_pool(name="const", bufs=1) as const,
            tc.tile_pool(name="sb", bufs=1) as sb,
            tc.tile_pool(name="workv", bufs=2) as workv,
            tc.tile_pool(name="small", bufs=1) as small,
        ):
            ident = const.tile([128, 128], BF16)
            make_identity(nc, ident)
            pools = (sb, workv, small)
            for rep in range(reps):
                _emit_body(nc, tc, pools, rep, q_d, p_d, qsum_d, diag_d, pos_d,
                           ident)

    nc.finalize()
    return nc


_NC_CACHE = []


def _get_nc():
    if not _NC_CACHE:
        _NC_CACHE.append(_build())
    return _NC_CACHE[0]


_RUNNER_CACHE = []


def _make_runner():
    """Build a cached jitted SPMD executor (mirrors bass2jax.run_bass_via_pjrt

    multi-core branch, but reusable across calls so repeat invocations skip
    recompilation)."""
    import jax
    from jax.experimental.shard_map import shard_map
    from jax.sharding import Mesh, PartitionSpec
    import concourse.mybir as _mybir
    from concourse import bass2jax

    nc = _get_nc()
    bass2jax.install_neuronx_cc_hook()

    partition_name = (
        nc.partition_id_tensor.name if nc.partition_id_tensor else None
    )
    in_names = []
    out_names = []
    out_avals = []
    zero_shapes = []
    for alloc in nc.m.functions[0].allocations:
        if not isinstance(alloc, _mybir.MemoryLocationSet):
            continue
        name = alloc.memorylocations[0].name
        if alloc.kind == "ExternalInput":
            if name != partition_name:
                in_names.append(name)
        elif alloc.kind == "ExternalOutput":
            out_names.append(name)
            shape = tuple(alloc.tensor_shape)
            dtype = _mybir.dt.np(alloc.dtype)
            out_avals.append(jax.core.ShapedArray(shape, dtype))
            zero_shapes.append((shape, dtype))
    n_params = len(in_names)
    n_outs = len(out_names)
    all_names = in_names + out_names
    if partition_name is not None:
        all_names = all_names + [partition_name]

    def _body(*args):
        operands = list(args)
        if partition_name is not None:
            operands.append(bass2jax.partition_id_tensor())
        outs = bass2jax._bass_exec_p.bind(
            *operands,
            out_avals=tuple(out_avals),
            in_names=tuple(all_names),
            out_names=tuple(out_names),
            lowering_input_output_aliases=(),
            sim_require_finite=True,
            sim_require_nnan=True,
            nc=nc,
        )
        return tuple(outs)

    devices = jax.devices()[:NCORES]
    mesh = Mesh(np.asarray(devices), ("core",))
    in_specs = (PartitionSpec("core"),) * (n_params + n_outs)
    out_specs = (PartitionSpec("core"),) * n_outs
    donate = tuple(range(n_params, n_params + n_outs))
    sharded = jax.jit(
        shard_map(
            _body, mesh=mesh, in_specs=in_specs, out_specs=out_specs,
            check_rep=False,
        ),
        donate_argnums=donate,
        keep_unused=True,
    )

    def run(in_maps):
        concat_in = [
            np.concatenate([np.asarray(in_maps[c][nm]) for c in range(NCORES)], axis=0)
            for nm in in_names
        ]
        concat_zeros = [
            np.zeros((NCORES * s[0], *s[1:]), dt) for s, dt in zero_shapes
        ]
        out_arrs = sharded(*concat_in, *concat_zeros)
        return [
            {
                nm: np.asarray(out_arrs[i]).reshape(NCORES, *out_avals[i].shape)[c]
                for i, nm in enumerate(out_names)
            }
            for c in range(NCORES)
        ]

    return run


def _get_runner():
    if not _RUNNER_CACHE:
        _RUNNER_CACHE.append(_make_runner())
    return _RUNNER_CACHE[0]


def _in_maps(x):
    q = x[:B]
    return [
        {
            "q": np.roll(q, -c * RPC, axis=0),
            "p": x[B + c * RPC : B + (c + 1) * RPC],
        }
        for c in range(NCORES)
    ]


def _reduce_results(results):
    total = np.float64(0.0)
    for r in results:
        denom = r["qsum"].astype(np.float64).sum(axis=-1) - r["diag"].astype(
            np.float64
        )
        pos = r["pos"].astype(np.float64)
        total += np.sum(np.log(denom) - pos / T_POS)
    return np.float32(total / B)


def _run(representations, **spmd_kwargs):
    x = np.ascontiguousarray(np.asarray(representations, dtype=np.float32))
    assert x.shape == (2 * B, D), x.shape
    nc = _get_nc()
    res = run_bass_kernel_spmd(
        nc, _in_maps(x), core_ids=list(range(NCORES)), **spmd_kwargs
    )
    return _reduce_results(res.results), res


def kernel(representations):
    x = np.ascontiguousarray(np.asarray(representations, dtype=np.float32))
    assert x.shape == (2 * B, D), x.shape
    results = _get_runner()(_in_maps(x))
    return _reduce_results(results)


if __name__ == "__main__":
    rng = np.random.default_rng(0)
    x = rng.standard_normal((2 * B, D), dtype=np.float32)
    print(kernel(x))

